# revision 26
# baseline (speedup 1.0000x reference)
"""Causal single-head attention on 8 trn2 cores, batch-data-parallel.

Computes, for each batch item b:
    Q = x[b] @ Wq + bq; K = x[b] @ Wk + bk; V = x[b] @ Wv + bv
    out[b] = softmax(causal_mask(Q K^T / sqrt(H))) @ V

Shapes: x [256, 256, 384], W* [384, 64], b* [64], out [256, 256, 64] fp32.
Sharding: batch axis split across 8 cores (32 items each), weights replicated.
The host feeds x transposed ([C, B*T] layout) so the kernel needs no on-device
transposition of x (contraction dim C must sit on SBUF partitions).

All matmul operands are float32r (TF32-like rounded fp32, ~1.6e-4 matmul rel
err, 4x PE throughput). Batch items are processed in pairs so the projection
matmuls stream N=512.

Per pair:
  qv psum [128,512] = rows 0:64 Q^T, rows 64:128 V^T (lhsT=[Wq|Wv], rhs=x^T)
  k  psum [64,512]  = K^T
  qv2/k2 sbuf = psum + per-partition bias ([bq;bv] and [bk]) -> one op each
Per batch item in the pair:
  V' = [V | 1 | 1] natural layout via PE transposes of V^T; the ones columns
       make the out matmul also produce the softmax denominator (wei @ 1)
  sT psum = scores^T: lhsT = K^T s-chunk, rhs = Q^T  (both h-major)
  W = exp(SCALE*sT + mask^T) -> written straight to SBUF, IS wei^T = out lhsT
  out' = wei^T.T @ V' -> [t, 64 | den | den];  out = out'[:,0:64] * (1/den)
"""

import numpy as np

import concourse.bacc as bacc
import concourse.mybir as mybir
import concourse.tile as tile
from concourse import bass_utils
from concourse.masks import make_identity

N_CORES = 8
B_FULL, T, C, H = 256, 256, 384, 64
B_SHARD = B_FULL // N_CORES  # 32
F32 = mybir.dt.float32
F32R = mybir.dt.float32r
SCALE = float(H) ** -0.5  # folded into exp: wei = exp(SCALE * scores + mask)
MASK_VAL = -1e30

ADD = mybir.AluOpType.add
MULT = mybir.AluOpType.mult
EXP = mybir.ActivationFunctionType.Exp


def _build():
    MMT = F32R
    nc = bacc.Bacc("TRN2", target_bir_lowering=False, debug=False, num_devices=N_CORES)

    # f32r-declared inputs: fp32 bit patterns fed directly; the PE's single-pass
    # fp32 mode consumes the high mantissa bits (same precision class as f32r
    # rounding, ~1e-4), and same-dtype DMA keeps loads on the fast HWDGE path.
    xT_d = nc.dram_tensor("xT", [C, B_SHARD * T], F32R, kind="ExternalInput").ap()
    wq_d = nc.dram_tensor("wq", [C, H], F32R, kind="ExternalInput").ap()
    wk_d = nc.dram_tensor("wk", [C, H], F32R, kind="ExternalInput").ap()
    wv_d = nc.dram_tensor("wv", [C, H], F32R, kind="ExternalInput").ap()
    bq_d = nc.dram_tensor("bq", [H, 1], F32, kind="ExternalInput").ap()
    bk_d = nc.dram_tensor("bk", [H, 1], F32, kind="ExternalInput").ap()
    bv_d = nc.dram_tensor("bv", [H, 1], F32, kind="ExternalInput").ap()
    out_d = nc.dram_tensor("out", [B_SHARD * T, H], F32, kind="ExternalOutput").ap()

    # x^T per batch pair: [p=c%128, k=c//128, t2=512]
    xT_r = xT_d.rearrange("(k p) (b t) -> b p k t", p=128, b=B_SHARD // 2)
    # out: t = n*128 + p per batch item
    out_r = out_d.rearrange("(b n p) h -> b p n h", p=128, n=2)

    with tile.TileContext(nc) as tc:
        with (
            tc.tile_pool(name="singles", bufs=1) as singles,
            tc.tile_pool(name="sb", bufs=3) as sb,
            tc.tile_pool(name="sbx", bufs=2) as sbx,
            tc.tile_pool(name="ps_qv", bufs=2, space="PSUM") as ps_qv,
            tc.tile_pool(name="ps_k", bufs=1, space="PSUM") as ps_k,
            tc.tile_pool(name="ps_s", bufs=2, space="PSUM") as ps_s,
            tc.tile_pool(name="ps_v", bufs=1, space="PSUM") as ps_v,
            tc.tile_pool(name="ps_o", bufs=2, space="PSUM") as ps_o,
        ):
            # ---- one-time setup ----
            identf = singles.tile([128, 128], F32)
            make_identity(nc, identf[:])
            ident = singles.tile([128, 128], MMT)
            nc.vector.tensor_copy(ident[:], identf[:])

            # mask for scores^T [s, t]: keep where t >= s, two diag blocks
            maskT2 = singles.tile([128, 2, 128], F32)
            nc.gpsimd.memset(maskT2[:], 0.0)
            for j in range(2):
                nc.gpsimd.affine_select(
                    out=maskT2[:, j, :],
                    in_=maskT2[:, j, :],
                    compare_op=mybir.AluOpType.is_ge,
                    fill=MASK_VAL,
                    base=0,
                    pattern=[[1, 128]],  # keep where (-s + t) >= 0
                    channel_multiplier=-1,
                )

            # [Wq | Wv] stacked along M; Wk zero-padded to M=128 (f32r matmuls
            # with partial column groups run in a slower mode).
            wqv = singles.tile([128, 3, 128], MMT)
            wkk = singles.tile([128, 3, 128], MMT)
            nc.vector.memset(wkk[:].bitcast(F32), 0.0)
            for c in range(3):
                nc.sync.dma_start(wqv[:, c, 0:64], wq_d[c * 128 : (c + 1) * 128, :])
                nc.sync.dma_start(wqv[:, c, 64:128], wv_d[c * 128 : (c + 1) * 128, :])
                nc.sync.dma_start(wkk[:, c, 0:64], wk_d[c * 128 : (c + 1) * 128, :])
            # per-partition bias vectors: [bq ; bv] and [bk]
            bqv_t = singles.tile([128, 1], F32)
            bk_t = singles.tile([64, 1], F32)
            nc.sync.dma_start(bqv_t[0:64, :], bq_d[:])
            nc.sync.dma_start(bqv_t[64:128, :], bv_d[:])
            nc.sync.dma_start(bk_t[:], bk_d[:])

            # HAM warmup: the PE clock-gate only opens (1.2 -> 2.4 GHz) after a
            # ~3.4us window of sustained matmul activity. Burn dummy matmuls
            # during the initial DMA wait so the real stream runs warm.
            wu = singles.tile([128, 256], MMT)
            nc.vector.memset(wu[:].bitcast(F32), 0.0)
            wu_ps = ps_s.tile([128, 256], F32, tag="s_ps")
            for _ in range(96):
                nc.tensor.matmul(wu_ps[:], wu[:, 0:128], wu[:], start=True, stop=True)

            for bp in range(B_SHARD // 2):
                # x^T for the pair (f32r bits straight off HWDGE)
                xt = sbx.tile([128, 3, 512], MMT, tag="xt")
                nc.sync.dma_start(xt[:], xT_r[bp])

                # pair projections
                qv_ps = ps_qv.tile([128, 512], F32, tag="qv_ps")
                k_ps = ps_k.tile([128, 512], F32, tag="k_ps")
                for c in range(3):
                    nc.tensor.matmul(
                        qv_ps[:], wqv[:, c, :], xt[:, c, :], start=(c == 0), stop=(c == 2)
                    )
                for c in range(3):
                    nc.tensor.matmul(
                        k_ps[:], wkk[:, c, :], xt[:, c, :], start=(c == 0), stop=(c == 2)
                    )
                qv2 = sb.tile([128, 512], MMT, tag="qv2")
                k2 = sb.tile([64, 512], MMT, tag="k2")
                nc.vector.tensor_scalar_add(qv2[:], qv_ps[:], bqv_t[:])
                nc.vector.tensor_scalar_add(k2[:], k_ps[0:64, :], bk_t[:])

                for bi in range(2):
                    toff = bi * 256
                    qT = qv2[0:64, toff : toff + 256]
                    kT = k2[0:64, toff : toff + 256]

                    # scores^T [s, t]: blocks [s0, t0:256] and [s1, t0:256]
                    s_ps = ps_s.tile([128, 4, 128], F32, tag="s_ps")
                    nc.tensor.matmul(s_ps[:, 0:2, :], kT[:, 0:128], qT, start=True, stop=True)
                    nc.tensor.matmul(s_ps[:, 2:4, :], kT[:, 128:256], qT, start=True, stop=True)

                    # V natural [s, h] + ones cols via PE transpose of V^T
                    v_ps = ps_v.tile([128, 2, 64], MMT, tag="v_ps")
                    for sh in range(2):
                        nc.tensor.transpose(
                            v_ps[:, sh, :],
                            qv2[64:128, toff + sh * 128 : toff + (sh + 1) * 128],
                            ident[64:128, 64:128],
                        )
                    v_sb = sb.tile([128, 2, 66], MMT, tag="v_sb")
                    nc.scalar.copy(v_sb[:, :, 0:64], v_ps[:])
                    nc.vector.memset(v_sb[:, :, 64:66].bitcast(F32), 1.0)

                    # wei^T = exp(SCALE*scores^T + mask), straight to SBUF
                    E = sb.tile([128, 2, 128], F32, tag="E")
                    W = sb.tile([128, 3, 128], MMT, tag="W")
                    nc.vector.tensor_add(E[:], s_ps[:, 0:4:3, :], maskT2[:])
                    nc.scalar.activation(W[:, 0:3:2, :], E[:], EXP, scale=SCALE)
                    nc.scalar.activation(W[:, 1, :], s_ps[:, 1, :], EXP, scale=SCALE)

                    # out' = wei^T.T @ [V|1|1] -> [t, 64 | den | den]
                    o_ps = ps_o.tile([128, 2, 66], F32, tag="o_ps")
                    nc.tensor.matmul(o_ps[:, 0, :], W[:, 0, :], v_sb[:, 0, :], start=True, stop=True)
                    nc.tensor.matmul(o_ps[:, 1, :], W[:, 1, :], v_sb[:, 0, :], start=True, stop=False)
                    nc.tensor.matmul(o_ps[:, 1, :], W[:, 2, :], v_sb[:, 1, :], start=False, stop=True)

                    rden = sb.tile([128, 2], F32, tag="rden")
                    nc.vector.reciprocal(rden[:], o_ps[:, :, 64])
                    o_sb = sb.tile([128, 2, 64], F32, tag="o_sb")
                    nc.vector.tensor_scalar_mul(o_sb[:, 0, :], o_ps[:, 0, 0:64], rden[:, 0:1])
                    nc.vector.tensor_scalar_mul(o_sb[:, 1, :], o_ps[:, 1, 0:64], rden[:, 1:2])
                    nc.sync.dma_start(out_r[bp * 2 + bi], o_sb[:])

    nc.compile()
    return nc


_CACHE = {}


def get_nc():
    if "nc" not in _CACHE:
        _CACHE["nc"] = _build()
    return _CACHE["nc"]


def make_in_maps(x, Wq, bq, Wk, bk, Wv, bv):
    x = np.asarray(x, dtype=np.float32)
    Wq = np.ascontiguousarray(np.asarray(Wq, dtype=np.float32))
    Wk = np.ascontiguousarray(np.asarray(Wk, dtype=np.float32))
    Wv = np.ascontiguousarray(np.asarray(Wv, dtype=np.float32))
    bq = np.ascontiguousarray(np.asarray(bq, dtype=np.float32)).reshape(H, 1)
    bk = np.ascontiguousarray(np.asarray(bk, dtype=np.float32)).reshape(H, 1)
    bv = np.ascontiguousarray(np.asarray(bv, dtype=np.float32)).reshape(H, 1)
    in_maps = []
    for i in range(N_CORES):
        shard = x[i * B_SHARD : (i + 1) * B_SHARD].reshape(B_SHARD * T, C)
        xT = np.ascontiguousarray(shard.T)  # [C, B_SHARD*T]
        in_maps.append(
            {"xT": xT, "wq": Wq, "wk": Wk, "wv": Wv, "bq": bq, "bk": bk, "bv": bv}
        )
    return in_maps


def kernel(x, Wq, bq, Wk, bk, Wv, bv):
    nc = get_nc()
    in_maps = make_in_maps(x, Wq, bq, Wk, bk, Wv, bv)
    res = bass_utils.run_bass_kernel_spmd(nc, in_maps, core_ids=list(range(N_CORES)))
    out = np.concatenate(
        [res.results[i]["out"].reshape(B_SHARD, T, H) for i in range(N_CORES)], axis=0
    )
    return out
